# revision 25
# baseline (speedup 1.0000x reference)
"""BasisResidualFFN Trainium2 kernel.

Math (per token t):
  recipe_soft = softmax(neuron_recipe, axis=-1)                 [64, 16]
  tr[t, :]    = sum_k w[t,k] * recipe_soft[idx[t,k], :]         [16]
  Y[t, (n,r)] = sum_d x[t,d] * basis_A[n,d,r]
  h[t, r]     = sum_n tr[t,n] * Y[t,(n,r)]
  delta[t, d] = sum_{n,r} basis_A[n,d,r] * tr[t,n] * h[t,r]
  out         = gelu((x + alpha*delta) @ w_up + b_up) @ w_down + b_down

Distribution: pure data parallel. B*S = 4096 tokens sharded 512/core
across 8 NeuronCores; all weights replicated. Everything on device is
computed feature-major (features on partitions, tokens on the free
axis) so no on-device activation transposes are needed; x arrives
pre-transposed from the host and the output is un-transposed there.

Precision: bf16 everywhere except the delta projection, which runs as
fp8 e4m3 DoubleRow matmuls (2x PE throughput): delta = (32*alpha*A2)^T
@ ct with ct in fp8 and the 1/32 compensation riding the PSUM drain
(x is preloaded into PSUM via a 32*identity matmul, so the drain is a
single scaled copy). delta errors enter only through alpha*delta with
alpha ~ 0.1, costing ~1e-4 extra rel err (measured 4e-3 total).

Schedule: the PE clock ramps over ~3us of continuous busy and
re-throttles after idle, so the PE must never stall. x and a1 stream
in dc-chunks and the YT matmuls run dc-outer, consuming each chunk as
it lands right behind the DMA; the routing matmuls (recsT, M_i, S^T
transposes, RepR) are interleaved between YT chunks so the tensor
queue stays dense while Vector runs the routing scatter. The FFN
weights stream behind, double-buffered, and the output leaves as bf16.
"""

import numpy as np

import concourse.bass as bass
import concourse.mybir as mybir
import concourse.tile as tile
from concourse import bacc
from concourse.bass import ts
from concourse.bass_utils import run_bass_kernel_spmd

P = 128
NCORES = 8
T = 512            # tokens per core
D = 1024
DFF = 4096
NB = 16            # n_basis
R = 32             # rank
NN = 64            # n_neurons
K = 8              # top-k
DC = D // P        # 8 contraction chunks over d
FT = DFF // P      # 32 ff tiles
DT = D // P        # 8 output d tiles
NRT = (NB * R) // P  # 4 (n,r) tiles
TT = T // P        # 4 token tiles per core

# const blob column layouts (bf16 blob / f32 blob)
BR_C, BR_ID, BR_ID32, BR_W = 0, 128, 256, 384
BF_BU, BF_BD, BF_REC, BF_W = 0, 32, 40, 56

F32 = mybir.dt.float32
BF16 = mybir.dt.bfloat16
F8 = mybir.dt.float8e4

DR = mybir.MatmulPerfMode.DoubleRow

NWARM = 6          # keeps the PE busy (and its clock ramped) until x lands
A2S = 32.0         # fp8 scale on alpha*A2; compensated in the xf drain
A1S = 16.0         # fp8 scale on A1; compensated in the C matrix

_BUILT = [None]


def _build_nc():
    nc = bacc.Bacc(None, target_bir_lowering=False)

    xtb_d = nc.dram_tensor("xtb", [P, DC, T], BF16, kind="ExternalInput")
    idxw_d = nc.dram_tensor("idxw", [P, TT, 2 * K], BF16, kind="ExternalInput")
    blobr_d = nc.dram_tensor("blobr", [P, BR_W], BF16, kind="ExternalInput")
    blobf_d = nc.dram_tensor("blobf", [P, BF_W], F32, kind="ExternalInput")
    sel_d = nc.dram_tensor("sel", [NB, NRT, P], BF16, kind="ExternalInput")
    a1_d = nc.dram_tensor("a1", [P, DC, NB * R], F8, kind="ExternalInput")
    a2_d = nc.dram_tensor("a2", [P, 2, 2, DT, P], F8, kind="ExternalInput")
    wu_d = nc.dram_tensor("wu", [FT // 2, P, 2, DC, P], BF16, kind="ExternalInput")
    wd_d = nc.dram_tensor("wd", [DT * 2, P, FT // 2, P], BF16, kind="ExternalInput")
    out_d = nc.dram_tensor("outT", [P, DT, T], BF16, kind="ExternalOutput")

    AF = mybir.ActivationFunctionType
    ALU = mybir.AluOpType

    with tile.TileContext(nc) as tc:
        with (
            tc.tile_pool(name="const", bufs=1) as constp,
            tc.tile_pool(name="smv", bufs=1) as smv,
            tc.tile_pool(name="small", bufs=2) as small,
            tc.tile_pool(name="stream", bufs=6) as stream,
            tc.tile_pool(name="wdstream", bufs=4) as wdstream,
            tc.tile_pool(name="otp", bufs=2) as otp,
            tc.tile_pool(name="psum", bufs=5, space="PSUM") as psum,
            tc.tile_pool(name="psumA", bufs=2, space="PSUM") as psumA,
            tc.tile_pool(name="psumB", bufs=1, space="PSUM") as psumB,
        ):
            # ---- DMA triggers, ordered by need-time. sync ring: routing
            # consts then x; scalar ring: a1 chunks then a2; gpsimd only
            # memsets (SWDGE is too slow for anything on the path) ----
            warm_sb = constp.tile([P, T], BF16, tag="warm")
            nc.gpsimd.memset(warm_sb[:], 0.0)
            # iota table for the scatter, generated on the (otherwise idle)
            # GpSimd lane instead of DMA'd: repeat(arange(64), 8)
            i512 = constp.tile([P, NN * K], BF16, tag="i512")
            nc.gpsimd.iota(i512[:], pattern=[[1, NN], [0, K]], base=0,
                           channel_multiplier=0,
                           allow_small_or_imprecise_dtypes=True)
            # a2 rides the slow SWDGE lane (needed only at ~18us)
            a2 = constp.tile([P, 2, 2, DT, P], F8, tag="a2")
            nc.gpsimd.dma_start(a2[:], a2_d[:])

            idxw = constp.tile([P, TT, 2 * K], BF16, tag="idxw")
            nc.sync.dma_start(idxw[:], idxw_d[:])
            sel = constp.tile([NB, NRT, P], BF16, tag="sel")
            nc.sync.dma_start(sel[:], sel_d[:])
            blobr = constp.tile([P, BR_W], BF16, tag="blobr")
            nc.sync.dma_start(blobr[:], blobr_d[:])
            # x split across the sync (dc 0-4) and scalar (dc 5-7) rings so
            # the two rings share the front stream
            xtb = constp.tile([P, DC, T], BF16, tag="xtb")
            nc.sync.dma_start(xtb[:, 0, :], xtb_d[:, 0, :])
            nc.sync.dma_start(xtb[:, 1:3, :], xtb_d[:, 1:3, :])
            nc.sync.dma_start(xtb[:, 3:5, :], xtb_d[:, 3:5, :])

            blobf = constp.tile([P, BF_W], F32, tag="blobf")
            nc.scalar.dma_start(blobf[:], blobf_d[:])
            a1 = constp.tile([P, DC, NB * R], F8, tag="a1f8")
            for hh in range(2):
                h4 = ts(hh, DC // 2)
                nc.scalar.dma_start(a1[:, h4, :], a1_d[:, h4, :])
            nc.scalar.dma_start(xtb[:, 5:8, :], xtb_d[:, 5:8, :])
            # exp of the recipe table early (needs only blobf)
            rec = blobf[:NN, BF_REC:BF_REC + NB]
            recsb = constp.tile([NN, NB], BF16, tag="recsb")
            ssum = small.tile([NN, 1], F32, tag="ssum")
            nc.scalar.activation(recsb[:], rec, AF.Exp, accum_out=ssum[:])
            # anchor read for the warm-up matmuls (prevents dead-code elim);
            # on Scalar so it cannot delay the Vector scatter or the PE
            warm_anchor = small.tile([P, 1], F32, tag="warm_anchor")

            bu = blobf[:, BF_BU:BF_BU + FT]
            bd = blobf[:, BF_BD:BF_BD + DT]
            cmat = blobr[:, BR_C:BR_C + P]
            identb = blobr[:, BR_ID:BR_ID + P]
            ident32 = blobr[:, BR_ID32:BR_ID32 + P]

            # ---- PE warm-up on the memset tile: starts at user-code time
            # zero with no DMA dependency so the clock ramp begins before the
            # first x/a1 chunk lands ----
            warm_ps = psumB.tile([P, T], F32, tag="b", name="warm")
            for w in range(NWARM):
                nc.tensor.matmul(warm_ps[:], warm_sb[:, :P], warm_sb[:],
                                 start=(w == 0), stop=(w == NWARM - 1))
            nc.scalar.activation(warm_anchor[:], warm_ps[:, 0:1], AF.Copy)

            # ---- routing scatter S[t, neuron] (weighted one-hot): one fused
            # 3-op Vector chain over all four token-tiles ----
            st_sb = constp.tile([NN, T], BF16, tag="st")
            iota4 = i512[:].rearrange(
                "p (o n k) -> p o n k", o=1, k=K).to_broadcast((P, TT, NN, K))
            idx_b = idxw[:, :, 0:K].rearrange(
                "p t (o k) -> p t o k", o=1).to_broadcast((P, TT, NN, K))
            w_b = idxw[:, :, K:2 * K].rearrange(
                "p t (o k) -> p t o k", o=1).to_broadcast((P, TT, NN, K))
            sk = smv.tile([P, TT, NN, K], BF16, tag="sk")
            s_all = smv.tile([P, TT, NN], BF16, tag="s")
            nc.vector.tensor_tensor(sk[:], iota4, idx_b, ALU.is_equal)
            nc.vector.tensor_tensor(sk[:], sk[:], w_b, ALU.mult)
            # pairwise-tree reduction over k (faster than reduce_sum on DVE)
            nc.vector.tensor_tensor(sk[:, :, :, 0:4], sk[:, :, :, 0:4],
                                    sk[:, :, :, 4:8], ALU.add)
            nc.vector.tensor_tensor(sk[:, :, :, 0:2], sk[:, :, :, 0:2],
                                    sk[:, :, :, 2:4], ALU.add)
            nc.vector.tensor_tensor(
                s_all[:].rearrange("p t (n o) -> p t n o", o=1),
                sk[:, :, :, 0:1], sk[:, :, :, 1:2], ALU.add)
            rsum = small.tile([NN, 1], F32, tag="rsum")
            nc.vector.reciprocal(rsum[:], ssum[:])

            yt_ps = [psum.tile([P, T], F32, tag="ps", name=f"yt{i}")
                     for i in range(NRT)]

            def yt_chunk(dc):
                for i in range(NRT):
                    nc.tensor.matmul(yt_ps[i][:], a1[:, dc, ts(i, P)],
                                     xtb[:, dc, :],
                                     start=(dc == 0), stop=(dc == DC - 1))

            # ---- YT chunks follow the x/a1 DMA; routing matmuls fill the
            # gaps between chunks ----
            yt_chunk(0)

            # recsT = recs^T, then M_i = (recs @ SEL_i)^T as [NN, P]
            rT_ps = psumB.tile([NB, NN], BF16, tag="b", name="rTps")
            nc.tensor.transpose(rT_ps[:], recsb[:], identb[:NN, :NN])
            recsT = constp.tile([NB, NN], BF16, tag="recsT")
            nc.scalar.activation(recsT[:], rT_ps[:], AF.Copy)

            # pre-issue the first three wu tiles on the scalar queue now, so
            # the up projection's weights are in flight long before the
            # scalar engine reaches the up loop
            wu_tiles = {}
            for ftp in range(3):
                wut = stream.tile([P, 2, DC, P], BF16, tag="wu",
                                  name=f"wu{ftp}")
                nc.scalar.dma_start(wut[:], wu_d[ftp])
                wu_tiles[ftp] = wut

            yt_chunk(1)

            m_sb = []
            for i in range(NRT):
                mp = psumA.tile([NN, P], F32, tag="rp", name=f"m{i}")
                nc.tensor.matmul(mp[:], recsT[:], sel[:, i, :],
                                 start=True, stop=True)
                ms = constp.tile([NN, P], BF16, tag=f"m{i}", name=f"ms{i}")
                nc.scalar.activation(ms[:], mp[:], AF.Copy)
                m_sb.append(ms)

            yt_chunk(2)
            yt_chunk(3)
            yt_chunk(4)

            # routing transposes: all four into one PSUM tile, one drain
            stp_all = psumB.tile([NN, TT * P], BF16, tag="b", name="stp")
            for tt in range(TT):
                nc.tensor.transpose(stp_all[:, ts(tt, P)], s_all[:, tt, :],
                                    identb)
            nc.scalar.activation(st_sb[:], stp_all[:], AF.Copy,
                                 scale=rsum[:, 0:1])

            yt_chunk(5)

            # RepR[i][(n,r), t] = tr[t, n(i,p)] = M_i^T @ S^T, into one tile
            reprall = constp.tile([P, NRT, T], BF16, tag="reprall")
            rr_ps = [psumA.tile([P, T], F32, tag="rp", name=f"rp{i}")
                     for i in range(NRT)]

            def repr_block(i):
                nc.tensor.matmul(rr_ps[i][:], m_sb[i][:], st_sb[:],
                                 start=True, stop=True)
                if i % 2 == 0:
                    nc.vector.tensor_copy(reprall[:, i, :], rr_ps[i][:])
                else:
                    nc.scalar.activation(reprall[:, i, :], rr_ps[i][:], AF.Copy)

            yt_chunk(6)
            repr_block(0)
            repr_block(1)
            yt_chunk(7)
            repr_block(2)
            repr_block(3)

            # ---- WYT = YT * RepR;  RepH = C^T @ sum-over-i WYT with C =
            # qred @ trep folded on the host (skips the ht intermediate);
            # the 32*x PSUM preloads fill PE gaps between the rh matmuls ----
            wyt = constp.tile([P, NRT, T], BF16, tag="wyt")
            xf = constp.tile([P, DC, T], BF16, tag="a1f8", name="xf")
            rh_ps = psumA.tile([P, T], F32, tag="rp", name="rh")
            dl_ps = {}
            for i in range(NRT):
                nc.vector.tensor_mul(out=wyt[:, i, :], in0=yt_ps[i][:],
                                     in1=reprall[:, i, :])
                nc.tensor.matmul(rh_ps[:], cmat, wyt[:, i, :],
                                 start=(i == 0), stop=(i == NRT - 1))
                dl_ps[i] = psum.tile([P, T], F32, tag="ps", name=f"dl{i}")
                nc.tensor.matmul(dl_ps[i][:], ident32, xtb[:, i, :],
                                 start=True, stop=False)
            dl_ps[4] = psum.tile([P, T], F32, tag="ps", name="dl4")
            nc.tensor.matmul(dl_ps[4][:], ident32, xtb[:, 4, :],
                             start=True, stop=False)

            # CT = RepH * RepR in fp8 (per-i so the first DoubleRow pair can
            # start early)
            ct = constp.tile([P, NRT, T], F8, tag="ct")
            for i in range(NRT):
                nc.vector.tensor_tensor(ct[:, i, :], rh_ps[:],
                                        reprall[:, i, :], ALU.mult)

            # ---- deltaT: fp8 DoubleRow, dt-outer so drains chase;  xf =
            # (32*x + 32*alpha*delta) / 32 on alternating Scalar/Vector ----
            def drain_xf(dt):
                if dt % 2 == 0:
                    nc.scalar.activation(xf[:, dt, :], dl_ps[dt][:], AF.Copy,
                                         scale=1.0 / A2S)
                else:
                    nc.vector.tensor_scalar_mul(xf[:, dt, :], dl_ps[dt][:],
                                                1.0 / A2S)

            def delta_block(dt):
                for pi in range(2):
                    nc.tensor.matmul(dl_ps[dt][:], a2[:, pi, :, dt, :],
                                     ct[:, 2 * pi:2 * pi + 2, :],
                                     start=False, stop=(pi == 1),
                                     perf_mode=DR)
                drain_xf(dt)

            for dt in range(3):
                delta_block(dt)
            for dt in range(5, DT):
                dl_ps[dt] = psum.tile([P, T], F32, tag="ps", name=f"dl{dt}")
                nc.tensor.matmul(dl_ps[dt][:], ident32, xtb[:, dt, :],
                                 start=True, stop=False)
                delta_block(dt - 2)
            delta_block(6)
            delta_block(7)

            # prefetch the first two wd tiles now (on the sync ring, which is
            # idle after x) so the down projection never waits on DMA at the
            # up->down boundary
            wd_tiles = {}
            for dt in range(2):
                for h in range(2):
                    wdt = wdstream.tile([P, FT // 2, P], BF16, tag="wd",
                                        name=f"wd{dt}_{h}")
                    nc.sync.dma_start(wdt[:], wd_d[dt * 2 + h])
                    wd_tiles[(dt, h)] = wdt

            # ---- FFN up + exact gelu; wu streams on the scalar ring so it
            # never competes with x/consts on the sync ring ----
            g = constp.tile([P, FT, T], BF16, tag="g")
            for ftp in range(FT // 2):
                if ftp in wu_tiles:
                    wu = wu_tiles.pop(ftp)
                else:
                    wu = stream.tile([P, 2, DC, P], BF16, tag="wu",
                                     name=f"wu{ftp}")
                    nc.scalar.dma_start(wu[:], wu_d[ftp])
                for j in range(2):
                    ft = 2 * ftp + j
                    u_ps = psum.tile([P, T], F32, tag="ps", name=f"u{ft}")
                    for dc in range(DC):
                        nc.tensor.matmul(u_ps[:], wu[:, j, dc, :], xf[:, dc, :],
                                         start=(dc == 0), stop=(dc == DC - 1))
                    nc.scalar.activation(g[:, ft, :], u_ps[:], AF.Gelu,
                                         bias=bu[:, ft:ft + 1], scale=1.0)

            # ---- FFN down + bias; bias-add split Vector/Scalar and the
            # output DMA split across two queues to shorten the tail ----
            TH = T // 2
            for dt in range(DT):
                o_ps = psum.tile([P, T], F32, tag="ps", name=f"o{dt}")
                for h in range(2):
                    wd = wd_tiles.pop((dt, h))
                    for fc in range(FT // 2):
                        fcg = h * (FT // 2) + fc
                        nc.tensor.matmul(o_ps[:], wd[:, fc, :], g[:, fcg, :],
                                         start=(fcg == 0), stop=(fcg == FT - 1))
                    if dt + 2 < DT:
                        wdt = wdstream.tile([P, FT // 2, P], BF16, tag="wd",
                                            name=f"wd{dt + 2}_{h}")
                        nc.sync.dma_start(wdt[:], wd_d[(dt + 2) * 2 + h])
                        wd_tiles[(dt + 2, h)] = wdt
                otv = otp.tile([P, TH], BF16, tag="otv", name=f"otv{dt}")
                nc.vector.tensor_scalar_add(otv[:], o_ps[:, 0:TH],
                                            bd[:, dt:dt + 1])
                nc.sync.dma_start(out_d[:, dt, 0:TH], otv[:])
                otg = otp.tile([P, TH], BF16, tag="otg", name=f"otg{dt}")
                nc.scalar.activation(otg[:], o_ps[:, TH:T], AF.Identity,
                                     bias=bd[:, dt:dt + 1], scale=1.0)
                nc.scalar.dma_start(out_d[:, dt, TH:T], otg[:])

    nc.finalize()
    return nc


def _get_nc():
    if _BUILT[0] is None:
        _BUILT[0] = _build_nc()
    return _BUILT[0]


def kernel(x, neuron_idx, neuron_weights, neuron_recipe, basis_A,
           w_up_w, w_up_b, w_down_w, w_down_b, alpha):
    import ml_dtypes
    nc = _get_nc()

    x = np.asarray(x, dtype=np.float32).reshape(NCORES * T, D)
    idxf = np.asarray(neuron_idx).astype(np.float32).reshape(NCORES * T, K)
    wgt = np.asarray(neuron_weights, dtype=np.float32).reshape(NCORES * T, K)
    rec = np.asarray(neuron_recipe, dtype=np.float32)
    bA = np.asarray(basis_A, dtype=np.float32)
    wu = np.asarray(w_up_w, dtype=np.float32)
    bu_in = np.asarray(w_up_b, dtype=np.float32)
    wd = np.asarray(w_down_w, dtype=np.float32)
    bd_in = np.asarray(w_down_b, dtype=np.float32)
    alpha_f = float(np.asarray(alpha, dtype=np.float32))

    # replicated operands, packed into the on-device layouts
    # A1 in fp8, scaled by 16 to stay in e4m3 normal range (1/16 in C)
    a1 = np.ascontiguousarray(
        np.clip(bA.transpose(1, 0, 2).reshape(D, NB * R) * A1S, -240.0, 240.0)
        .reshape(DC, P, NB * R).transpose(1, 0, 2)
    ).astype(ml_dtypes.float8_e4m3)
    # delta projection in fp8: 32*alpha*A2, [(pair, j, p), (dt, c)] packed
    a2m = np.clip(bA.transpose(0, 2, 1).reshape(NB * R, D) * (alpha_f * A2S),
                  -240.0, 240.0)
    a2 = np.ascontiguousarray(
        a2m.reshape(2, 2, P, DT, P).transpose(2, 0, 1, 3, 4)
    ).astype(ml_dtypes.float8_e4m3)
    wu_p = np.ascontiguousarray(
        wu.reshape(DC, P, FT // 2, 2, P).transpose(2, 1, 3, 0, 4)
    ).astype(ml_dtypes.bfloat16)
    wd_p = np.ascontiguousarray(
        wd.reshape(2, FT // 2, P, DT, P).transpose(3, 0, 2, 1, 4)
        .reshape(DT * 2, P, FT // 2, P)).astype(ml_dtypes.bfloat16)

    blobf = np.zeros((P, BF_W), dtype=np.float32)
    blobf[:, BF_BU:BF_BU + FT] = bu_in.reshape(FT, P).T
    blobf[:, BF_BD:BF_BD + DT] = bd_in.reshape(DT, P).T
    blobf[:NN, BF_REC:BF_REC + NB] = rec

    blobr = np.zeros((P, BR_W), dtype=np.float32)
    # C = qred @ trep fused: C[q, p] = 1/A1S iff q % R == p % R
    blobr[:, BR_C:BR_C + P] = (
        np.arange(P)[:, None] % R == np.arange(P)[None, :] % R) / A1S
    blobr[:, BR_ID:BR_ID + P] = np.eye(P, dtype=np.float32)
    blobr[:, BR_ID32:BR_ID32 + P] = np.eye(P, dtype=np.float32) * A2S
    blobr = blobr.astype(ml_dtypes.bfloat16)

    # SEL[n, i, m] = 1 iff n in [4i, 4i+4) and m // 32 == n - 4i
    sel = np.zeros((NB, NRT, P), dtype=np.float32)
    for n in range(NB):
        i, nloc = divmod(n, NRT)
        sel[n, i, nloc * R:(nloc + 1) * R] = 1.0
    sel = sel.astype(ml_dtypes.bfloat16)

    shared = {
        "blobf": blobf, "blobr": blobr, "sel": sel, "a1": a1, "a2": a2,
        "wu": wu_p, "wd": wd_p,
    }
    in_maps = []
    idxw = np.concatenate([idxf, wgt], axis=1).astype(
        ml_dtypes.bfloat16)  # [N*T, 16]
    for c in range(NCORES):
        xc = x[c * T:(c + 1) * T]  # [T, D]
        xtc = np.ascontiguousarray(xc.T.reshape(DC, P, T).transpose(1, 0, 2))
        xtbc = xtc.astype(ml_dtypes.bfloat16)
        iwc = np.ascontiguousarray(
            idxw[c * T:(c + 1) * T].reshape(TT, P, 2 * K).transpose(1, 0, 2))
        in_maps.append({"xtb": xtbc, "idxw": iwc, **shared})

    res = run_bass_kernel_spmd(nc, in_maps, core_ids=list(range(NCORES)))

    out = np.empty((NCORES * T, D), dtype=np.float32)
    for c in range(NCORES):
        ot = res.results[c]["outT"].astype(np.float32)  # [P, DT, T]
        out[c * T:(c + 1) * T] = ot.transpose(1, 0, 2).reshape(D, T).T
    return out.reshape(2, 2048, D)
